# revision 28
# baseline (speedup 1.0000x reference)
"""DotInteraction Trainium2 kernel.

features [16384, 27, 128] f32 -> strict-lower-triangle pairwise dots [16384, 351].

Pure data parallel over batch: 2048 samples per core on 8 cores; each core
computes its samples' 27x27 Gram matrices on the PE and ships the blocks
back; the host gathers the tril indices.

Design (evolved 151.8us -> 77.2us through trace-driven iterations):

  1. fp16 everywhere off-chip: inputs quantized on host (Gram accumulation
     stays fp32 in PSUM; max rel err 4.9e-4 vs the 2e-2 gate), halving DMA
     bytes and running the PE at 1 cycle/row instead of fp32's 4. (int8
     would halve input again and passes the error gate at 1.3e-2, but the
     PE matmul only accepts fp32/bf16/fp16/fp8; fp8 fails the gate at
     ~5e-2 -- both measured/simulated.)
  2. Host pre-transposes and pads: xt [128(D), 512 quartets, 4 samples, 32]
     fp16 (cols 27-31 zero). A quartet's 128 cols are contiguous -> ONE
     matmul per quartet with M=128 contiguous weights (triggers Fast
     Weight Load); sample j's Gram block lands at partitions 32j (engine
     APs require partition bases in {0,32,64,96}; BIR verifier rejects
     anything else, so dense 27-col packing is impossible -- the stationary
     operand AP must also be single-free-dim, which kills overlapping-
     window tricks). rhs streams 26 cols/sample (N=104; Gram col 26 is
     never needed by the strict lower triangle).
  3. Whole-core input resident in one SBUF tile, filled by upfront partial
     DMAs: small head partials so compute starts early, 2MB bodies for
     DMA efficiency, small tail partials so the last round isn't gated on
     a big transfer. ~305GB/s aggregate observed (HBM-bound; per-NC cap
     ~358 shared with the output stream).
  4. PSUM scatter layout [128, 8 banks, 512] f32: round r -> bank r%8;
     quartet s's block j goes to cols 104j+26s (matmul out AP has free
     dims (4,26) stride (104,1)), so the j-band of one round is 104
     contiguous cols. Extraction every 2 rounds: per band j one copy
     [27p, 2, 104] PSUM->gs fp16, alternating vector/scalar 8:7. Copy
     cost is ~205ns fixed (PSUM access latency) + 0.71ns/col; 2-bank
     groups keep 3 groups of write-after-read lookahead (4-bank groups
     measured 20-27us slower from PE stalls on bank recycling).
  5. 16 output chunks of the 3.41MB gs staging buffer: the first 9 via
     gpsimd.dma_start (SWDGE/"HAM" queue, ~32-49GB/s -- slow but runs in
     parallel with the input stream and adds real bandwidth; all-sync and
     all-HAM variants both measured slower), the last 7 split into halves
     via nc.sync.dma_start (HWDGE -> the 16 fast queues, which are idle
     by then) so the final drain is short. Band-split (108-partition)
     outputs measured slower: 45kB DMAs are descriptor-dominated.

Per-core budget at 2.4GHz PE / ~305GB/s queue + ~32GB/s HAM DMA: 20.2MB
total wire ~60us, boot-to-first-byte ~9.4us, PE busy ~43us (hidden under
DMA), extraction ~45us/engine (hidden), drain ~2-5us.
"""
import numpy as np

B, F, D = 16384, 27, 128
NCORES = 8
BL = B // NCORES            # samples per core (2048)
NQ = BL // 4                # quartets per core (512)
NR = NQ // 4                # rounds (128), 4 quartets per round
NG = NR // 2                # extraction groups (64), 8 quartets each
INP = 32                    # input partial DMAs
OUTP = 16                   # output chunks (tail drained via sync/HWDGE)
HAMP = 9                    # output chunks on the gpsimd/SWDGE path
SP = NQ * 128               # padded xt cols per core (65536)
CC = F - 1                  # cols kept per Gram block (tril j<=25)
GC = NQ * CC                # gs cols (13312)

_CACHE = {}


def _build():
    import concourse.tile as tile
    from concourse import bacc, mybir
    from concourse.ap import AP

    f16 = mybir.dt.float16
    f32 = mybir.dt.float32
    nc = bacc.Bacc("TRN2", target_bir_lowering=False, debug=False)
    feat = nc.dram_tensor("features", [D, SP], f16, kind="ExternalInput")
    out_d = nc.dram_tensor("out", [128, GC], f16, kind="ExternalOutput")

    # input partial col sizes: small first partials for early compute
    # start, 1MB (4096-col) bodies for DMA efficiency, small tail chunks
    # to keep the last-round latency low
    IWS = ([512, 2048] + [4096] * 14 + [2048, 1024, 1024, 512, 512, 512])
    assert sum(IWS) == SP
    OW = GC // OUTP         # gs cols per output DMA (1728)

    with tile.TileContext(nc) as tc:
        with (
            tc.tile_pool(name="xt", bufs=1) as xt_pool,
            tc.tile_pool(name="gs", bufs=1) as gs_pool,
            tc.tile_pool(name="pg", bufs=1, space="PSUM") as pg_pool,
        ):
            xt = xt_pool.tile([128, NQ, 4, 32], f16)
            gs = gs_pool.tile([128, NQ, CC], f16)
            xf = xt[:].rearrange("p q s c -> p (q s c)")
            o = 0
            for w in IWS:
                nc.sync.dma_start(xf[:, o:o + w], feat[:, o:o + w])
                o += w

            # PSUM scatter layout: round r -> bank r%8; within a bank,
            # quartet s's Gram block j lands at cols 104j + 26s, so the
            # j-band of one round is 104 CONTIGUOUS cols (4 quartets x 26).
            # Extraction runs 104-col segments instead of 8x 26-col ones
            # (DVE/ACT copy cost is ~26ns/segment + 0.71ns/col).
            pg = pg_pool.tile([128, 8, 512], f32)
            pgt = pg[:].tensor
            PP = 8 * 512        # psum partition pitch (f32 elements)
            gf = gs[:].rearrange("p q c -> p (q c)")
            for g in range(NG):
                b0 = (2 * g) % 8
                for r in range(2 * g, 2 * g + 2):
                    for s in range(4):
                        Q = 4 * r + s
                        # rhs 26 cols/sample: Gram col 26 is never
                        # extracted (strict lower tri needs cols 0..25)
                        out_ap = AP(pgt, 512 * (r % 8) + 26 * s,
                                    [(PP, 128), (104, 4), (1, 26)])
                        nc.tensor.matmul(
                            out_ap,
                            xt[:, Q, :, :],
                            xt[:, Q, :, 0:CC],
                        )
                if g < NG - 1:
                    for j in range(4):
                        src = AP(pgt, 32 * j * PP + 512 * b0 + 104 * j,
                                 [(PP, F), (512, 2), (1, 104)])
                        dst = gs[32 * j:32 * j + F, 8 * g:8 * (g + 1), :
                                 ].rearrange("p (b s) c -> p b (s c)",
                                             b=2, s=4)
                        if ((4 * g + j) * 8) % 15 < 8:
                            nc.vector.tensor_copy(dst, src)
                        else:
                            nc.scalar.copy(dst, src)
                else:
                    # last group: extract per bank so the final-round
                    # latency after the last input chunk is halved
                    for b in range(2):
                        for j in range(4):
                            src = AP(pgt,
                                     32 * j * PP + 512 * (b0 + b) + 104 * j,
                                     [(PP, F), (1, 104)])
                            q0 = 8 * g + 4 * b
                            dst = gs[32 * j:32 * j + F, q0:q0 + 4, :
                                     ].rearrange("p s c -> p (s c)")
                            if (2 * b + j) % 2 == 0:
                                nc.vector.tensor_copy(dst, src)
                            else:
                                nc.scalar.copy(dst, src)

                if g % (NG // OUTP) == NG // OUTP - 1:
                    op = g // (NG // OUTP)
                    if op < HAMP:
                        c0 = OW // 2 if op == 0 else OW * op
                        nc.gpsimd.dma_start(
                            out_d[:, c0:OW * (op + 1)],
                            gf[:, c0:OW * (op + 1)])
                    else:
                        # drain tail on the fast sync/HWDGE queues (idle
                        # once input issue is done), in small halves
                        h = OW // 2
                        for k in range(2):
                            c0 = OW * op + h * k
                            nc.sync.dma_start(
                                out_d[:, c0:c0 + h], gf[:, c0:c0 + h])
                elif g == 1:
                    # early half-chunk on the HAM path: starts the slow
                    # SWDGE stream ~2us sooner
                    nc.gpsimd.dma_start(out_d[:, 0:OW // 2],
                                        gf[:, 0:OW // 2])

    nc.compile()
    return nc


def _run_spmd(nc, in_maps):
    """Like bass2jax.run_bass_via_pjrt multi-core, but builds the global
    sharded arrays from per-device shards (device_put per core) instead of
    one host concat — a single large host->device transfer can fail on the
    axon relay; per-core transfers are fine."""
    import jax
    from jax.experimental.shard_map import shard_map
    from jax.sharding import Mesh, NamedSharding, PartitionSpec
    from concourse import bass2jax, mybir

    bass2jax.install_neuronx_cc_hook()
    partition_name = (nc.partition_id_tensor.name
                      if nc.partition_id_tensor else None)
    in_names, out_names, out_avals = [], [], []
    for alloc in nc.m.functions[0].allocations:
        if not isinstance(alloc, mybir.MemoryLocationSet):
            continue
        name = alloc.memorylocations[0].name
        if alloc.kind == "ExternalInput":
            if name != partition_name:
                in_names.append(name)
        elif alloc.kind == "ExternalOutput":
            out_names.append(name)
            out_avals.append(jax.core.ShapedArray(
                tuple(alloc.tensor_shape), mybir.dt.np(alloc.dtype)))
    n_params = len(in_names)
    n_outs = len(out_names)
    all_in_names = list(in_names) + list(out_names)
    if partition_name is not None:
        all_in_names.append(partition_name)

    def _body(*args):
        operands = list(args)
        if partition_name is not None:
            operands.append(bass2jax.partition_id_tensor())
        outs = bass2jax._bass_exec_p.bind(
            *operands,
            out_avals=tuple(out_avals),
            in_names=tuple(all_in_names),
            out_names=tuple(out_names),
            lowering_input_output_aliases=(),
            sim_require_finite=True,
            sim_require_nnan=True,
            nc=nc,
        )
        return tuple(outs)

    devices = jax.devices()[:NCORES]
    mesh = Mesh(np.asarray(devices), ("core",))
    sharding = NamedSharding(mesh, PartitionSpec("core"))
    donate = tuple(range(n_params, n_params + n_outs))
    sharded = jax.jit(
        shard_map(_body, mesh=mesh,
                  in_specs=(PartitionSpec("core"),) * (n_params + n_outs),
                  out_specs=(PartitionSpec("core"),) * n_outs,
                  check_rep=False),
        donate_argnums=donate, keep_unused=True)

    def _global(per_core):
        shards = [jax.device_put(per_core[c], devices[c])
                  for c in range(NCORES)]
        gshape = (NCORES * per_core[0].shape[0], *per_core[0].shape[1:])
        return jax.make_array_from_single_device_arrays(
            gshape, sharding, shards)

    gins = [_global([np.asarray(m[name]) for m in in_maps])
            for name in in_names]
    gzeros = [_global([np.zeros(av.shape, av.dtype)] * NCORES)
              for av in out_avals]
    out_arrs = sharded(*gins, *gzeros)

    fetched = [np.asarray(a).reshape(NCORES, *out_avals[i].shape)
               for i, a in enumerate(out_arrs)]
    return [{name: fetched[i][c] for i, name in enumerate(out_names)}
            for c in range(NCORES)]


def kernel(features: np.ndarray) -> np.ndarray:
    features = np.asarray(features, dtype=np.float32)
    assert features.shape == (B, F, D), features.shape

    if "nc" not in _CACHE:
        _CACHE["nc"] = _build()
    nc = _CACHE["nc"]

    # [B, F, D] -> fp16 -> per-core padded X^T [D, NQ, 4, 32]
    f16 = features.astype(np.float16)
    xp = np.zeros((NCORES, D, NQ, 4, 32), dtype=np.float16)
    xp[..., :F] = f16.reshape(NCORES, NQ, 4, F, D).transpose(0, 4, 1, 2, 3)
    xp = xp.reshape(NCORES, D, SP)
    in_maps = [{"features": xp[c]} for c in range(NCORES)]

    results = _run_spmd(nc, in_maps)

    # [NCORES][128, 13312] fp16: [32j+r, 26Q + c] = G_{4Q+j}[r, c], c<26
    dump = np.stack([r["out"] for r in results])          # [8, 128, 13312]
    v = dump.reshape(NCORES, 128, NQ, CC)                 # [c, p, Q, col]
    G = np.empty((NCORES, NQ, 4, F, CC), dtype=np.float16)
    for j in range(4):
        G[:, :, j] = v[:, 32 * j:32 * j + F].transpose(0, 2, 1, 3)
    G = G.reshape(B, F, CC)

    rows, cols = np.tril_indices(F, k=-1)
    return G[:, rows, cols].astype(np.float32)



# revision 29
# speedup vs baseline: 1.1524x; 1.1524x over previous
"""DotInteraction Trainium2 kernel.

features [16384, 27, 128] f32 -> strict-lower-triangle pairwise dots [16384, 351].

Pure data parallel over batch: 2048 samples per core on 8 cores; each core
computes its samples' 27x27 Gram matrices on the PE and ships the blocks
back; the host gathers the tril indices.

Design (evolved 151.8us -> 77.2us through trace-driven iterations):

  1. fp16 everywhere off-chip: inputs quantized on host (Gram accumulation
     stays fp32 in PSUM; max rel err 4.9e-4 vs the 2e-2 gate), halving DMA
     bytes and running the PE at 1 cycle/row instead of fp32's 4. (int8
     would halve input again and passes the error gate at 1.3e-2, but the
     PE matmul only accepts fp32/bf16/fp16/fp8; fp8 fails the gate at
     ~5e-2 -- both measured/simulated.)
  2. Host pre-transposes and pads: xt [128(D), 512 quartets, 4 samples, 32]
     fp16 (cols 27-31 zero). A quartet's 128 cols are contiguous -> ONE
     matmul per quartet with M=128 contiguous weights (triggers Fast
     Weight Load); sample j's Gram block lands at partitions 32j (engine
     APs require partition bases in {0,32,64,96}; BIR verifier rejects
     anything else, so dense 27-col packing is impossible -- the stationary
     operand AP must also be single-free-dim, which kills overlapping-
     window tricks). rhs streams 26 cols/sample (N=104; Gram col 26 is
     never needed by the strict lower triangle).
  3. Whole-core input resident in one SBUF tile, filled by upfront partial
     DMAs: small head partials so compute starts early, 2MB bodies for
     DMA efficiency, small tail partials so the last round isn't gated on
     a big transfer. ~305GB/s aggregate observed (HBM-bound; per-NC cap
     ~358 shared with the output stream).
  4. PSUM scatter layout [128, 8 banks, 512] f32: round r -> bank r%8;
     quartet s's block j goes to cols 104j+26s (matmul out AP has free
     dims (4,26) stride (104,1)), so the j-band of one round is 104
     contiguous cols. Extraction every 2 rounds: per band j one copy
     [27p, 2, 104] PSUM->gs fp16, alternating vector/scalar 8:7. Copy
     cost is ~205ns fixed (PSUM access latency) + 0.71ns/col; 2-bank
     groups keep 3 groups of write-after-read lookahead (4-bank groups
     measured 20-27us slower from PE stalls on bank recycling).
  5. 16 output chunks of the 3.41MB gs staging buffer: the first 9 via
     gpsimd.dma_start (SWDGE/"HAM" queue, ~32-49GB/s -- slow but runs in
     parallel with the input stream and adds real bandwidth; all-sync and
     all-HAM variants both measured slower), the last 7 split into halves
     via nc.sync.dma_start (HWDGE -> the 16 fast queues, which are idle
     by then) so the final drain is short. Band-split (108-partition)
     outputs measured slower: 45kB DMAs are descriptor-dominated.

Per-core budget at 2.4GHz PE / ~305GB/s queue + ~32GB/s HAM DMA: 20.2MB
total wire ~60us, boot-to-first-byte ~9.4us, PE busy ~43us (hidden under
DMA), extraction ~45us/engine (hidden), drain ~2-5us.
"""
import numpy as np

B, F, D = 16384, 27, 128
NCORES = 8
BL = B // NCORES            # samples per core (2048)
NQ = BL // 4                # quartets per core (512)
NR = NQ // 4                # rounds (128), 4 quartets per round
NG = NR // 2                # extraction groups (64), 8 quartets each
INP = 32                    # input partial DMAs
OUTP = 16                   # output chunks (tail drained via sync/HWDGE)
HAMP = 9                    # output chunks on the gpsimd/SWDGE path
SP = NQ * 128               # padded xt cols per core (65536)
CC = F - 1                  # cols kept per Gram block (tril j<=25)
GC = NQ * CC                # gs cols (13312)

_CACHE = {}


def _build():
    import concourse.tile as tile
    from concourse import bacc, mybir
    from concourse.ap import AP

    f16 = mybir.dt.float16
    f32 = mybir.dt.float32
    nc = bacc.Bacc("TRN2", target_bir_lowering=False, debug=False)
    feat = nc.dram_tensor("features", [D, SP], f16, kind="ExternalInput")
    out_d = nc.dram_tensor("out", [128, GC], f16, kind="ExternalOutput")

    # input partial col sizes: small first partials for early compute
    # start, 1MB (4096-col) bodies for DMA efficiency, small tail chunks
    # to keep the last-round latency low
    IWS = ([512, 2048] + [4096] * 14 + [2048, 1024, 1024, 512, 512, 512])
    assert sum(IWS) == SP
    OW = GC // OUTP         # gs cols per output DMA (1728)

    with tile.TileContext(nc) as tc:
        with (
            tc.tile_pool(name="xt", bufs=1) as xt_pool,
            tc.tile_pool(name="gs", bufs=1) as gs_pool,
            tc.tile_pool(name="pg", bufs=1, space="PSUM") as pg_pool,
        ):
            xt = xt_pool.tile([128, NQ, 4, 32], f16)
            gs = gs_pool.tile([128, NQ, CC], f16)
            xf = xt[:].rearrange("p q s c -> p (q s c)")
            o = 0
            for w in IWS:
                nc.sync.dma_start(xf[:, o:o + w], feat[:, o:o + w])
                o += w

            # PSUM scatter layout: round r -> bank r%8; within a bank,
            # quartet s's Gram block j lands at cols 104j + 26s, so the
            # j-band of one round is 104 CONTIGUOUS cols (4 quartets x 26).
            # Extraction runs 104-col segments instead of 8x 26-col ones
            # (DVE/ACT copy cost is ~26ns/segment + 0.71ns/col).
            pg = pg_pool.tile([128, 8, 512], f32)
            pgt = pg[:].tensor
            PP = 8 * 512        # psum partition pitch (f32 elements)
            gf = gs[:].rearrange("p q c -> p (q c)")
            for g in range(NG):
                b0 = (2 * g) % 8
                for r in range(2 * g, 2 * g + 2):
                    for s in range(4):
                        Q = 4 * r + s
                        # rhs 26 cols/sample: Gram col 26 is never
                        # extracted (strict lower tri needs cols 0..25)
                        out_ap = AP(pgt, 512 * (r % 8) + 26 * s,
                                    [(PP, 128), (104, 4), (1, 26)])
                        nc.tensor.matmul(
                            out_ap,
                            xt[:, Q, :, :],
                            xt[:, Q, :, 0:CC],
                        )
                for j in range(4):
                    src = AP(pgt, 32 * j * PP + 512 * b0 + 104 * j,
                             [(PP, F), (512, 2), (1, 104)])
                    dst = gs[32 * j:32 * j + F, 8 * g:8 * (g + 1), :
                             ].rearrange("p (b s) c -> p b (s c)", b=2, s=4)
                    if ((4 * g + j) * 8) % 15 < 8:
                        nc.vector.tensor_copy(dst, src)
                    else:
                        nc.scalar.copy(dst, src)

                if g % (NG // OUTP) == NG // OUTP - 1:
                    op = g // (NG // OUTP)
                    if op < HAMP:
                        nc.gpsimd.dma_start(
                            out_d[:, OW * op:OW * (op + 1)],
                            gf[:, OW * op:OW * (op + 1)])
                    else:
                        # drain tail on the fast sync/HWDGE queues (idle
                        # once input issue is done), in small halves
                        h = OW // 2
                        for k in range(2):
                            c0 = OW * op + h * k
                            nc.sync.dma_start(
                                out_d[:, c0:c0 + h], gf[:, c0:c0 + h])

    nc.compile()
    return nc


def _run_spmd(nc, in_maps):
    """Like bass2jax.run_bass_via_pjrt multi-core, but builds the global
    sharded arrays from per-device shards (device_put per core) instead of
    one host concat — a single large host->device transfer can fail on the
    axon relay; per-core transfers are fine."""
    import jax
    from jax.experimental.shard_map import shard_map
    from jax.sharding import Mesh, NamedSharding, PartitionSpec
    from concourse import bass2jax, mybir

    bass2jax.install_neuronx_cc_hook()
    partition_name = (nc.partition_id_tensor.name
                      if nc.partition_id_tensor else None)
    in_names, out_names, out_avals = [], [], []
    for alloc in nc.m.functions[0].allocations:
        if not isinstance(alloc, mybir.MemoryLocationSet):
            continue
        name = alloc.memorylocations[0].name
        if alloc.kind == "ExternalInput":
            if name != partition_name:
                in_names.append(name)
        elif alloc.kind == "ExternalOutput":
            out_names.append(name)
            out_avals.append(jax.core.ShapedArray(
                tuple(alloc.tensor_shape), mybir.dt.np(alloc.dtype)))
    n_params = len(in_names)
    n_outs = len(out_names)
    all_in_names = list(in_names) + list(out_names)
    if partition_name is not None:
        all_in_names.append(partition_name)

    def _body(*args):
        operands = list(args)
        if partition_name is not None:
            operands.append(bass2jax.partition_id_tensor())
        outs = bass2jax._bass_exec_p.bind(
            *operands,
            out_avals=tuple(out_avals),
            in_names=tuple(all_in_names),
            out_names=tuple(out_names),
            lowering_input_output_aliases=(),
            sim_require_finite=True,
            sim_require_nnan=True,
            nc=nc,
        )
        return tuple(outs)

    devices = jax.devices()[:NCORES]
    mesh = Mesh(np.asarray(devices), ("core",))
    sharding = NamedSharding(mesh, PartitionSpec("core"))
    donate = tuple(range(n_params, n_params + n_outs))
    sharded = jax.jit(
        shard_map(_body, mesh=mesh,
                  in_specs=(PartitionSpec("core"),) * (n_params + n_outs),
                  out_specs=(PartitionSpec("core"),) * n_outs,
                  check_rep=False),
        donate_argnums=donate, keep_unused=True)

    def _global(per_core):
        shards = [jax.device_put(per_core[c], devices[c])
                  for c in range(NCORES)]
        gshape = (NCORES * per_core[0].shape[0], *per_core[0].shape[1:])
        return jax.make_array_from_single_device_arrays(
            gshape, sharding, shards)

    gins = [_global([np.asarray(m[name]) for m in in_maps])
            for name in in_names]
    gzeros = [_global([np.zeros(av.shape, av.dtype)] * NCORES)
              for av in out_avals]
    out_arrs = sharded(*gins, *gzeros)

    fetched = [np.asarray(a).reshape(NCORES, *out_avals[i].shape)
               for i, a in enumerate(out_arrs)]
    return [{name: fetched[i][c] for i, name in enumerate(out_names)}
            for c in range(NCORES)]


def kernel(features: np.ndarray) -> np.ndarray:
    features = np.asarray(features, dtype=np.float32)
    assert features.shape == (B, F, D), features.shape

    if "nc" not in _CACHE:
        _CACHE["nc"] = _build()
    nc = _CACHE["nc"]

    # [B, F, D] -> fp16 -> per-core padded X^T [D, NQ, 4, 32]
    f16 = features.astype(np.float16)
    xp = np.zeros((NCORES, D, NQ, 4, 32), dtype=np.float16)
    xp[..., :F] = f16.reshape(NCORES, NQ, 4, F, D).transpose(0, 4, 1, 2, 3)
    xp = xp.reshape(NCORES, D, SP)
    in_maps = [{"features": xp[c]} for c in range(NCORES)]

    results = _run_spmd(nc, in_maps)

    # [NCORES][128, 13312] fp16: [32j+r, 26Q + c] = G_{4Q+j}[r, c], c<26
    dump = np.stack([r["out"] for r in results])          # [8, 128, 13312]
    v = dump.reshape(NCORES, 128, NQ, CC)                 # [c, p, Q, col]
    G = np.empty((NCORES, NQ, 4, F, CC), dtype=np.float16)
    for j in range(4):
        G[:, :, j] = v[:, 32 * j:32 * j + F].transpose(0, 2, 1, 3)
    G = G.reshape(B, F, CC)

    rows, cols = np.tril_indices(F, k=-1)
    return G[:, rows, cols].astype(np.float32)



# revision 30
# speedup vs baseline: 1.1740x; 1.0188x over previous
"""DotInteraction Trainium2 kernel.

features [16384, 27, 128] f32 -> strict-lower-triangle pairwise dots [16384, 351].

Pure data parallel over batch: 2048 samples per core on 8 cores; each core
computes its samples' 27x27 Gram matrices on the PE and ships the blocks
back; the host gathers the tril indices.

Design (evolved 151.8us -> 77.2us through trace-driven iterations):

  1. fp16 everywhere off-chip: inputs quantized on host (Gram accumulation
     stays fp32 in PSUM; max rel err 4.9e-4 vs the 2e-2 gate), halving DMA
     bytes and running the PE at 1 cycle/row instead of fp32's 4. (int8
     would halve input again and passes the error gate at 1.3e-2, but the
     PE matmul only accepts fp32/bf16/fp16/fp8; fp8 fails the gate at
     ~5e-2 -- both measured/simulated.)
  2. Host pre-transposes and pads: xt [128(D), 512 quartets, 4 samples, 32]
     fp16 (cols 27-31 zero). A quartet's 128 cols are contiguous -> ONE
     matmul per quartet with M=128 contiguous weights (triggers Fast
     Weight Load); sample j's Gram block lands at partitions 32j (engine
     APs require partition bases in {0,32,64,96}; BIR verifier rejects
     anything else, so dense 27-col packing is impossible -- the stationary
     operand AP must also be single-free-dim, which kills overlapping-
     window tricks). rhs streams 26 cols/sample (N=104; Gram col 26 is
     never needed by the strict lower triangle).
  3. Whole-core input resident in one SBUF tile, filled by upfront partial
     DMAs: small head partials so compute starts early, 1MB bodies for
     DMA efficiency, small tail partials so the last round isn't gated on
     a big transfer. ~305GB/s aggregate observed (HBM-bound; per-NC cap
     ~358 shared with the output stream; 2MB bodies measured slower).
  4. PSUM scatter layout [128, 8 banks, 512] f32: round r -> bank r%8;
     quartet s's block j goes to cols 104j+26s (matmul out AP has free
     dims (4,26) stride (104,1)), so the j-band of one round is 104
     contiguous cols. Extraction every 2 rounds: per band j one copy
     [27p, 2, 104] PSUM->gs fp16, alternating vector/scalar 8:7. Copy
     cost is ~205ns fixed (PSUM access latency) + 0.71ns/col; 2-bank
     groups keep 3 groups of write-after-read lookahead (4-bank groups
     measured 20-27us slower from PE stalls on bank recycling).
  5. 16 output chunks of the 3.41MB gs staging buffer: the first 9 via
     gpsimd.dma_start (SWDGE/"HAM" queue, ~32-49GB/s -- slow but runs in
     parallel with the input stream and adds real bandwidth; all-sync and
     all-HAM variants both measured slower), the last 7 split into halves
     via nc.sync.dma_start (HWDGE -> the 16 fast queues, which are idle
     by then) so the final drain is short. Band-split (108-partition)
     outputs measured slower: 45kB DMAs are descriptor-dominated.

Per-core budget at 2.4GHz PE / ~305GB/s queue + ~32GB/s HAM DMA: 20.2MB
total wire ~60us, boot-to-first-byte ~9.4us, PE busy ~43us (hidden under
DMA), extraction ~45us/engine (hidden), drain ~2-5us.
"""
import numpy as np

B, F, D = 16384, 27, 128
NCORES = 8
BL = B // NCORES            # samples per core (2048)
NQ = BL // 4                # quartets per core (512)
NR = NQ // 4                # rounds (128), 4 quartets per round
NG = NR // 2                # extraction groups (64), 8 quartets each
INP = 32                    # input partial DMAs
OUTP = 16                   # output chunks (tail drained via sync/HWDGE)
HAMP = 9                    # output chunks on the gpsimd/SWDGE path
SP = NQ * 128               # padded xt cols per core (65536)
CC = F - 1                  # cols kept per Gram block (tril j<=25)
GC = NQ * CC                # gs cols (13312)

_CACHE = {}


def _build():
    import concourse.tile as tile
    from concourse import bacc, mybir
    from concourse.ap import AP

    f16 = mybir.dt.float16
    f32 = mybir.dt.float32
    nc = bacc.Bacc("TRN2", target_bir_lowering=False, debug=False)
    feat = nc.dram_tensor("features", [D, SP], f16, kind="ExternalInput")
    out_d = nc.dram_tensor("out", [128, GC], f16, kind="ExternalOutput")

    # input partial col sizes: small first partials for early compute
    # start, 1MB (4096-col) bodies for DMA efficiency, small tail chunks
    # to keep the last-round latency low
    IWS = ([512, 2048] + [4096] * 14 + [2048, 1024, 1024, 512, 512, 512])
    assert sum(IWS) == SP
    OW = GC // OUTP         # gs cols per output DMA (1728)

    with tile.TileContext(nc) as tc:
        with (
            tc.tile_pool(name="xt", bufs=1) as xt_pool,
            tc.tile_pool(name="gs", bufs=1) as gs_pool,
            tc.tile_pool(name="pg", bufs=1, space="PSUM") as pg_pool,
        ):
            xt = xt_pool.tile([128, NQ, 4, 32], f16)
            gs = gs_pool.tile([128, NQ, CC], f16)
            xf = xt[:].rearrange("p q s c -> p (q s c)")
            o = 0
            for w in IWS:
                nc.sync.dma_start(xf[:, o:o + w], feat[:, o:o + w])
                o += w

            # PSUM scatter layout: round r -> bank r%8; within a bank,
            # quartet s's Gram block j lands at cols 104j + 26s, so the
            # j-band of one round is 104 CONTIGUOUS cols (4 quartets x 26).
            # Extraction runs 104-col segments instead of 8x 26-col ones
            # (DVE/ACT copy cost is ~26ns/segment + 0.71ns/col).
            pg = pg_pool.tile([128, 8, 512], f32)
            pgt = pg[:].tensor
            PP = 8 * 512        # psum partition pitch (f32 elements)
            gf = gs[:].rearrange("p q c -> p (q c)")
            for g in range(NG):
                b0 = (2 * g) % 8
                for r in range(2 * g, 2 * g + 2):
                    for s in range(4):
                        Q = 4 * r + s
                        # rhs 26 cols/sample: Gram col 26 is never
                        # extracted (strict lower tri needs cols 0..25)
                        out_ap = AP(pgt, 512 * (r % 8) + 26 * s,
                                    [(PP, 128), (104, 4), (1, 26)])
                        nc.tensor.matmul(
                            out_ap,
                            xt[:, Q, :, :],
                            xt[:, Q, :, 0:CC],
                        )
                for j in range(4):
                    src = AP(pgt, 32 * j * PP + 512 * b0 + 104 * j,
                             [(PP, F), (512, 2), (1, 104)])
                    dst = gs[32 * j:32 * j + F, 8 * g:8 * (g + 1), :
                             ].rearrange("p (b s) c -> p b (s c)", b=2, s=4)
                    if ((4 * g + j) * 8) % 15 < 8:
                        nc.vector.tensor_copy(dst, src)
                    else:
                        nc.scalar.copy(dst, src)

                if g % (NG // OUTP) == NG // OUTP - 1:
                    op = g // (NG // OUTP)
                    if op < HAMP:
                        nc.gpsimd.dma_start(
                            out_d[:, OW * op:OW * (op + 1)],
                            gf[:, OW * op:OW * (op + 1)])
                    else:
                        # drain tail on the fast sync/HWDGE queues (idle
                        # once input issue is done), in small halves
                        h = OW // 2
                        for k in range(2):
                            c0 = OW * op + h * k
                            nc.sync.dma_start(
                                out_d[:, c0:c0 + h], gf[:, c0:c0 + h])

    nc.compile()
    return nc


def _run_spmd(nc, in_maps):
    """Like bass2jax.run_bass_via_pjrt multi-core, but builds the global
    sharded arrays from per-device shards (device_put per core) instead of
    one host concat — a single large host->device transfer can fail on the
    axon relay; per-core transfers are fine."""
    import jax
    from jax.experimental.shard_map import shard_map
    from jax.sharding import Mesh, NamedSharding, PartitionSpec
    from concourse import bass2jax, mybir

    bass2jax.install_neuronx_cc_hook()
    partition_name = (nc.partition_id_tensor.name
                      if nc.partition_id_tensor else None)
    in_names, out_names, out_avals = [], [], []
    for alloc in nc.m.functions[0].allocations:
        if not isinstance(alloc, mybir.MemoryLocationSet):
            continue
        name = alloc.memorylocations[0].name
        if alloc.kind == "ExternalInput":
            if name != partition_name:
                in_names.append(name)
        elif alloc.kind == "ExternalOutput":
            out_names.append(name)
            out_avals.append(jax.core.ShapedArray(
                tuple(alloc.tensor_shape), mybir.dt.np(alloc.dtype)))
    n_params = len(in_names)
    n_outs = len(out_names)
    all_in_names = list(in_names) + list(out_names)
    if partition_name is not None:
        all_in_names.append(partition_name)

    def _body(*args):
        operands = list(args)
        if partition_name is not None:
            operands.append(bass2jax.partition_id_tensor())
        outs = bass2jax._bass_exec_p.bind(
            *operands,
            out_avals=tuple(out_avals),
            in_names=tuple(all_in_names),
            out_names=tuple(out_names),
            lowering_input_output_aliases=(),
            sim_require_finite=True,
            sim_require_nnan=True,
            nc=nc,
        )
        return tuple(outs)

    devices = jax.devices()[:NCORES]
    mesh = Mesh(np.asarray(devices), ("core",))
    sharding = NamedSharding(mesh, PartitionSpec("core"))
    donate = tuple(range(n_params, n_params + n_outs))
    sharded = jax.jit(
        shard_map(_body, mesh=mesh,
                  in_specs=(PartitionSpec("core"),) * (n_params + n_outs),
                  out_specs=(PartitionSpec("core"),) * n_outs,
                  check_rep=False),
        donate_argnums=donate, keep_unused=True)

    def _global(per_core):
        shards = [jax.device_put(per_core[c], devices[c])
                  for c in range(NCORES)]
        gshape = (NCORES * per_core[0].shape[0], *per_core[0].shape[1:])
        return jax.make_array_from_single_device_arrays(
            gshape, sharding, shards)

    gins = [_global([np.asarray(m[name]) for m in in_maps])
            for name in in_names]
    gzeros = [_global([np.zeros(av.shape, av.dtype)] * NCORES)
              for av in out_avals]
    out_arrs = sharded(*gins, *gzeros)

    fetched = [np.asarray(a).reshape(NCORES, *out_avals[i].shape)
               for i, a in enumerate(out_arrs)]
    return [{name: fetched[i][c] for i, name in enumerate(out_names)}
            for c in range(NCORES)]


def kernel(features: np.ndarray) -> np.ndarray:
    features = np.asarray(features, dtype=np.float32)
    assert features.shape == (B, F, D), features.shape

    if "nc" not in _CACHE:
        _CACHE["nc"] = _build()
    nc = _CACHE["nc"]

    # [B, F, D] -> fp16 -> per-core padded X^T [D, NQ, 4, 32]
    f16 = features.astype(np.float16)
    xp = np.zeros((NCORES, D, NQ, 4, 32), dtype=np.float16)
    xp[..., :F] = f16.reshape(NCORES, NQ, 4, F, D).transpose(0, 4, 1, 2, 3)
    xp = xp.reshape(NCORES, D, SP)
    in_maps = [{"features": xp[c]} for c in range(NCORES)]

    results = _run_spmd(nc, in_maps)

    # [NCORES][128, 13312] fp16: [32j+r, 26Q + c] = G_{4Q+j}[r, c], c<26
    dump = np.stack([r["out"] for r in results])          # [8, 128, 13312]
    v = dump.reshape(NCORES, 128, NQ, CC)                 # [c, p, Q, col]
    G = np.empty((NCORES, NQ, 4, F, CC), dtype=np.float16)
    for j in range(4):
        G[:, :, j] = v[:, 32 * j:32 * j + F].transpose(0, 2, 1, 3)
    G = G.reshape(B, F, CC)

    rows, cols = np.tril_indices(F, k=-1)
    return G[:, rows, cols].astype(np.float32)

